# revision 6
# baseline (speedup 1.0000x reference)
"""Fused GroupNorm + attention block for Trainium2 (8 NeuronCores, SPMD).

Strategy:
  - Each core computes the full output for 1/8 of the spatial positions
    (a 512-column slice of the flattened [C=256, N=4096] activation).
  - GroupNorm is folded into the QKV weights on-device:
      qkv = (W diag(a)) x + W b~   with a[c] = rstd[g(c)]*gamma[c],
      b~[c] = beta[c] - mean[g(c)]*a[c].
    The attention logit scale (1/8) is folded into the q columns of W^T
    on the host. The k bias drops out of softmax exactly; the v bias is
    recovered through the softmax row-sums and folded into the output
    bias B* = b_proj + W_p beta_v.
  - Attention is computed in "S^T" layout: S^T[s,t] = sum_d k[d,s] q[d,t]
    so the exp'd probabilities feed the AV matmul directly (contraction
    over s on the partition axis, no transposes). Row-sums come from an
    extra all-ones column appended to v^T.
  - Hot-path matmuls run in bf16 (activations are O(1) after the GN fold,
    measured end-to-end error ~3e-4 relative).
"""

import numpy as np

import concourse.bass as bass
import concourse.bacc as bacc
import concourse.tile as tile
import concourse.mybir as mybir
from concourse.bass_utils import run_bass_kernel_spmd

F32 = mybir.dt.float32
BF16 = mybir.dt.bfloat16
I32 = mybir.dt.int32
AF = mybir.ActivationFunctionType
OP = mybir.AluOpType

C = 256
N = 4096
NCORES = 8
TSL = N // NCORES          # 512 spatial positions per core
HEADS = 4
D = 64                     # head dim
NG = 16                    # groupnorm groups
GS = C // NG               # channels per group
EPS = 1e-5
NSB = N // 128             # 32 s-blocks
VW = D + 1                 # v^T columns per head incl. ones column


def _build():
    nc = bacc.Bacc("TRN2", target_bir_lowering=False, debug=False,
                   num_devices=NCORES)

    x_d = nc.dram_tensor("x2", [2, 128, N], F32, kind="ExternalInput")
    xq_d = nc.dram_tensor("xq", [2, 128, TSL], F32, kind="ExternalInput")
    wqkvT_d = nc.dram_tensor("wqkvT", [2, 128, 3 * C], F32, kind="ExternalInput")
    wprojT_d = nc.dram_tensor("wprojT", [2, 128, C], F32, kind="ExternalInput")
    gamma_d = nc.dram_tensor("gamma_col", [2, 128, 1], F32, kind="ExternalInput")
    beta_d = nc.dram_tensor("beta_col", [2, 128, 1], F32, kind="ExternalInput")
    bproj_d = nc.dram_tensor("bproj_col", [2, 128, 1], F32, kind="ExternalInput")
    out_d = nc.dram_tensor("out", [2, 128, TSL], F32, kind="ExternalOutput")

    with tile.TileContext(nc) as tc:
        _emit(nc, tc, x_d, xq_d, wqkvT_d, wprojT_d, gamma_d, beta_d,
              bproj_d, out_d)
    nc.finalize()
    return nc


def _emit(nc, tc, x_d, xq_d, wqkvT_d, wprojT_d, gamma_d, beta_d, bproj_d,
          out_d):
    import contextlib
    ctx = contextlib.ExitStack()
    with ctx:
        CP = ctx.enter_context(tc.tile_pool(name="const", bufs=1))
        WK = ctx.enter_context(tc.tile_pool(name="work", bufs=2))
        PS = ctx.enter_context(tc.tile_pool(name="psum", bufs=1, space="PSUM"))
        PPool = ctx.enter_context(tc.tile_pool(name="ptiles", bufs=3))

        # ---------------- loads (x in 8 pieces for stats pipelining) ------
        # x pieces: [c-tile][p] each [128, 1024], p = quarter of the N axis
        xch = [[CP.tile([128, N // 4], F32, tag=f"x{ct}{p}", name=f"x{ct}{p}")
                for p in range(4)] for ct in range(2)]
        xb = [[CP.tile([128, N // 4], BF16, tag=f"xb{ct}{p}", name=f"xb{ct}{p}")
               for p in range(4)] for ct in range(2)]
        stats = [WK.tile([128, 8, 6], F32, tag=f"bnstats{ct}", bufs=1,
                         name=f"bnstats{ct}") for ct in range(2)]
        for p in range(4):
            for ct in range(2):
                nc.sync.dma_start(out=xch[ct][p],
                                  in_=x_d[ct, :, p * (N // 4):(p + 1) * (N // 4)])
        xq = [CP.tile([128, TSL], F32, tag=f"xq{ct}", name=f"xq{ct}")
              for ct in range(2)]
        xqb = [CP.tile([128, TSL], BF16, tag=f"xqb{ct}", name=f"xqb{ct}")
               for ct in range(2)]
        wqkvT = [CP.tile([128, 3 * C], F32, tag=f"wq{ct}", name=f"wq{ct}")
                 for ct in range(2)]
        wprojT = [CP.tile([128, C], F32, tag=f"wp{ct}", name=f"wp{ct}")
                  for ct in range(2)]
        gcol = [CP.tile([128, 1], F32, tag=f"g{ct}", name=f"g{ct}") for ct in range(2)]
        bcol = [CP.tile([128, 1], F32, tag=f"b{ct}", name=f"b{ct}") for ct in range(2)]
        bpcol = [CP.tile([128, 1], F32, tag=f"bp{ct}", name=f"bp{ct}") for ct in range(2)]
        for ct in range(2):
            nc.sync.dma_start(out=xq[ct], in_=xq_d[ct])
            nc.sync.dma_start(out=wqkvT[ct], in_=wqkvT_d[ct])
            nc.sync.dma_start(out=wprojT[ct], in_=wprojT_d[ct])
            nc.sync.dma_start(out=gcol[ct], in_=gamma_d[ct])
            nc.sync.dma_start(out=bcol[ct], in_=beta_d[ct])
            nc.sync.dma_start(out=bpcol[ct], in_=bproj_d[ct])

        # per-piece: bn_stats (DVE) + bf16 cast (ACT), overlapping the DMAs
        for p in range(4):
            for ct in range(2):
                xv = xch[ct][p].rearrange("q (j f) -> q j f", f=512)
                for j in range(2):
                    nc.vector.bn_stats(out=stats[ct][:, 2 * p + j, :],
                                       in_=xv[:, j, :])
                nc.scalar.copy(out=xb[ct][p], in_=xch[ct][p])
        for ct in range(2):
            nc.scalar.copy(out=xqb[ct], in_=xq[ct])

        # ---------------- groupnorm statistics ----------------
        # indicator matrices built from iota + shift + is_equal
        gci = WK.tile([128, 1], I32, tag="gci", bufs=1, name="gci")
        nc.gpsimd.iota(gci, pattern=[[0, 1]], base=0, channel_multiplier=1)
        nc.vector.tensor_scalar(out=gci, in0=gci, scalar1=4, scalar2=None,
                                op0=OP.arith_shift_right)
        gcf = WK.tile([128, 1], F32, tag="gcf", bufs=1, name="gcf")
        nc.vector.tensor_copy(out=gcf, in_=gci)
        ji = WK.tile([128, 8], I32, tag="ji", bufs=1, name="ji")
        nc.gpsimd.iota(ji, pattern=[[1, 8]], base=0, channel_multiplier=0)
        jf = WK.tile([128, 8], F32, tag="jf", bufs=1, name="jf")
        nc.vector.tensor_copy(out=jf, in_=ji)
        G = CP.tile([128, 8], F32, tag="G", name="G")
        nc.vector.tensor_scalar(out=G, in0=jf, scalar1=gcf, scalar2=None,
                                op0=OP.is_equal)
        ci = WK.tile([8, 128], I32, tag="ci", bufs=1, name="ci")
        nc.gpsimd.iota(ci, pattern=[[1, 128]], base=0, channel_multiplier=0)
        nc.vector.tensor_scalar(out=ci, in0=ci, scalar1=4, scalar2=None,
                                op0=OP.arith_shift_right)
        cf = WK.tile([8, 128], F32, tag="cf", bufs=1, name="cf")
        nc.vector.tensor_copy(out=cf, in_=ci)
        gi = WK.tile([8, 1], I32, tag="gi", bufs=1, name="gi")
        nc.gpsimd.iota(gi, pattern=[[0, 1]], base=0, channel_multiplier=1)
        gif = WK.tile([8, 1], F32, tag="gif", bufs=1, name="gif")
        nc.vector.tensor_copy(out=gif, in_=gi)
        Gt = CP.tile([8, 128], F32, tag="Gt", name="Gt")
        nc.vector.tensor_scalar(out=Gt, in0=cf, scalar1=gif, scalar2=None,
                                op0=OP.is_equal)
        eps8 = CP.tile([8, 1], F32, tag="eps8", name="eps8")
        nc.vector.memset(eps8, EPS)
        ones16 = CP.tile([128, 16], F32, tag="ones16", name="ones16")
        nc.vector.memset(ones16, 1.0)

        # per-channel packed [mean, E[x^2]]
        mvp = [CP.tile([128, 2], F32, tag=f"mvp{ct}", name=f"mvp{ct}")
               for ct in range(2)]
        for ct in range(2):
            mv = WK.tile([128, 2], F32, tag="bnaggr", bufs=2, name="bnaggr")
            nc.vector.bn_aggr(out=mv, in_=stats[ct])
            nc.vector.tensor_copy(out=mvp[ct][:, 0:1], in_=mv[:, 0:1])
            nc.vector.tensor_tensor(out=mvp[ct][:, 1:2], in0=mv[:, 0:1],
                                    in1=mv[:, 0:1], op=OP.mult)
            nc.vector.tensor_tensor(out=mvp[ct][:, 1:2], in0=mvp[ct][:, 1:2],
                                    in1=mv[:, 1:2], op=OP.add)

        # group sums via G^T @ mvp -> gg[g, 2*ct + j]
        gg = PS.tile([8, 4], F32, tag="S", bufs=2, name="S")
        for ct in range(2):
            nc.tensor.matmul(gg[:, 2 * ct:2 * ct + 2], lhsT=G, rhs=mvp[ct],
                             start=(ct == 0), stop=(ct == 1))
        ggv = gg.rearrange("p (ct two) -> p ct two", two=2)
        meanL = CP.tile([8, 2], F32, tag="meanL", name="meanL")
        rstd = CP.tile([8, 2], F32, tag="rstd", name="rstd")
        tmp8 = WK.tile([8, 2], F32, tag="tmp8", name="tmp8")
        nc.vector.tensor_scalar(out=meanL, in0=ggv[:, :, 0], scalar1=1.0 / GS,
                                scalar2=None, op0=OP.mult)
        nc.vector.tensor_scalar(out=tmp8, in0=ggv[:, :, 1], scalar1=1.0 / GS,
                                scalar2=None, op0=OP.mult)
        nc.vector.tensor_tensor(out=rstd, in0=meanL, in1=meanL, op=OP.mult)
        nc.vector.tensor_tensor(out=rstd, in0=tmp8, in1=rstd, op=OP.subtract)
        # rstd = exp(-0.5 * ln(var + eps))
        nc.scalar.activation(out=rstd, in_=rstd, func=AF.Ln, bias=eps8)
        nc.scalar.activation(out=rstd, in_=rstd, func=AF.Exp, scale=-0.5)

        # expand group values to channels and build a, b~
        acol = [CP.tile([128, 1], F32, tag=f"acol{ct}", name=f"acol{ct}")
                for ct in range(2)]
        btcol = [CP.tile([128, 1], F32, tag=f"btcol{ct}", name=f"btcol{ct}")
                 for ct in range(2)]
        for ct in range(2):
            rexp = PS.tile([128, 1], F32, tag="S", bufs=2, name="S")
            nc.tensor.matmul(rexp, lhsT=Gt, rhs=rstd[:, ct:ct + 1],
                             start=True, stop=True)
            mexp = PS.tile([128, 1], F32, tag="S", bufs=2, name="S")
            nc.tensor.matmul(mexp, lhsT=Gt, rhs=meanL[:, ct:ct + 1],
                             start=True, stop=True)
            nc.vector.tensor_tensor(out=acol[ct], in0=rexp, in1=gcol[ct],
                                    op=OP.mult)
            nc.vector.tensor_tensor(out=btcol[ct], in0=mexp, in1=acol[ct],
                                    op=OP.mult)
            nc.vector.tensor_tensor(out=btcol[ct], in0=bcol[ct], in1=btcol[ct],
                                    op=OP.subtract)

        # scaled weights W'^T = W^T * a (per-partition), bf16
        wqs = [CP.tile([128, 3 * C], BF16, tag=f"wqs{ct}", name=f"wqs{ct}")
               for ct in range(2)]
        for ct in range(2):
            nc.vector.tensor_scalar_mul(out=wqs[ct], in0=wqkvT[ct],
                                        scalar1=acol[ct])

        # qkv bias beta = W^T.T @ b~  (q blocks 0,1 and v blocks 4,5)
        betaq = CP.tile([128, 2], F32, tag="betaq", name="betaq")
        betav = CP.tile([128, 2], F32, tag="betav", name="betav")
        for i, ob in enumerate((0, 1, 4, 5)):
            bps = PS.tile([128, 1], F32, tag="S", bufs=2, name="S")
            for ct in range(2):
                nc.tensor.matmul(bps, lhsT=wqkvT[ct][:, 128 * ob:128 * (ob + 1)],
                                 rhs=btcol[ct], start=(ct == 0), stop=(ct == 1))
            dst = betaq if ob < 2 else betav
            nc.vector.tensor_copy(out=dst[:, i % 2:i % 2 + 1], in_=bps)

        # B* = b_proj + W_p @ beta_v
        bstar = CP.tile([128, 2], F32, tag="bstar", name="bstar")
        for ob in range(2):
            bps = PS.tile([128, 1], F32, tag="S", bufs=2, name="S")
            for ct in range(2):
                nc.tensor.matmul(bps,
                                 lhsT=wprojT[ct][:, 128 * ob:128 * (ob + 1)],
                                 rhs=betav[:, ct:ct + 1],
                                 start=(ct == 0), stop=(ct == 1))
            nc.vector.tensor_tensor(out=bstar[:, ob:ob + 1], in0=bps,
                                    in1=bpcol[ob], op=OP.add)

        # per-head projection weights at partitions 0-63 (DMA partition shift)
        wps4 = [CP.tile([64, C], BF16, tag=f"wps4_{h}", name=f"wps4_{h}")
                for h in range(HEADS)]
        wpsb = [CP.tile([128, C], BF16, tag=f"wpsb{ct}", name=f"wpsb{ct}")
                for ct in range(2)]
        for ct in range(2):
            nc.vector.tensor_copy(out=wpsb[ct], in_=wprojT[ct])
        for h in range(HEADS):
            if h % 2 == 0:
                nc.vector.tensor_copy(out=wps4[h], in_=wpsb[h // 2][0:64, :])
            else:
                nc.sync.dma_start(out=wps4[h], in_=wpsb[h // 2][64:128, :])

        # ---------------- q (this core's slice) ----------------
        q = [CP.tile([128, TSL], BF16, tag=f"q{ob}", name=f"q{ob}")
             for ob in range(2)]
        for ob in range(2):
            qps = PS.tile([128, TSL], F32, tag="S", bufs=2, name="S")
            for ct in range(2):
                nc.tensor.matmul(qps,
                                 lhsT=wqs[ct][:, 128 * ob:128 * (ob + 1)],
                                 rhs=xqb[ct], start=(ct == 0), stop=(ct == 1))
            nc.vector.tensor_scalar(out=q[ob], in0=qps,
                                    scalar1=betaq[:, ob:ob + 1], scalar2=None,
                                    op0=OP.add)

        # ---------------- k, v^T production helpers ----------------
        kc = [[CP.tile([128, 512], BF16, tag=f"k{ob}_{j}", name=f"k{ob}_{j}")
               for j in range(8)] for ob in range(2)]
        vt = [CP.tile([128, 4, 4 * VW], BF16, tag=f"vt{j}", name=f"vt{j}")
              for j in range(8)]

        def xb_slice(j, ct, width, off=0):
            # bf16 x slice covering s in [512j + off, 512j + off + width)
            p = j // 2
            col = 512 * (j % 2) + off
            return xb[ct][p][:, col:col + width]

        def produce_k(j, ob):
            kps = PS.tile([128, 512], F32, tag="S", bufs=2, name="S")
            for ct in range(2):
                nc.tensor.matmul(
                    kps,
                    lhsT=wqs[ct][:, C + 128 * ob: C + 128 * (ob + 1)],
                    rhs=xb_slice(j, ct, 512), start=(ct == 0), stop=(ct == 1))
            nc.vector.tensor_copy(out=kc[ob][j], in_=kps)

        def produce_v(j):
            vps = PS.tile([128, 4, C], F32, tag="S", bufs=2, name="S")
            for jj in range(4):
                for ct in range(2):
                    nc.tensor.matmul(
                        vps[:, jj, :],
                        lhsT=xb_slice(j, ct, 128, off=128 * jj),
                        rhs=wqs[ct][:, 2 * C:3 * C],
                        start=(ct == 0), stop=(ct == 1))
            vtv = vt[j].rearrange("p j (h w) -> p j h w", w=VW)
            nc.vector.tensor_copy(
                out=vtv[:, :, :, 0:D],
                in_=vps.rearrange("p j (h d) -> p j h d", d=D))
            nc.vector.tensor_copy(
                out=vtv[:, :, :, D:VW],
                in_=ones16.rearrange("p (j h w) -> p j h w", h=4, w=1))

        hp = [PS.tile([VW, TSL], F32, tag=f"h{h}", name=f"h{h}")
              for h in range(HEADS)]

        # ---------------- main attention loop ----------------
        produce_k(0, 0)
        produce_k(0, 1)
        produce_v(0)
        for j in range(8):
            for jj in range(4):
                for pair in range(2):
                    sps = PS.tile([128, 2 * TSL], F32, tag="S", bufs=2, name="S")
                    for po in range(2):
                        base = 64 * po
                        nc.tensor.matmul(
                            sps[:, TSL * po:TSL * (po + 1)],
                            lhsT=kc[pair][j][base:base + 64,
                                             128 * jj:128 * (jj + 1)],
                            rhs=q[pair][base:base + 64, :],
                            start=True, stop=True)
                    pt = PPool.tile([128, 2 * TSL], BF16, tag="P", name="P")
                    nc.scalar.activation(out=pt, in_=sps, func=AF.Exp)
                    sb = 4 * j + jj
                    for po in range(2):
                        h = 2 * pair + po
                        nc.tensor.matmul(
                            hp[h],
                            lhsT=vt[j][:, jj, VW * h:VW * (h + 1)],
                            rhs=pt[:, TSL * po:TSL * (po + 1)],
                            start=(sb == 0), stop=(sb == NSB - 1))
                # prefetch next chunk's k/v between attention steps
                if j < 7:
                    if jj == 1:
                        produce_k(j + 1, 0)
                    elif jj == 2:
                        produce_k(j + 1, 1)
                    elif jj == 3:
                        produce_v(j + 1)

        # ---------------- softmax normalization + projection ----------------
        rs = [WK.tile([128, TSL], F32, tag="rs", bufs=4, name="rs")
              for _ in range(HEADS)]
        bb = [WK.tile([64, TSL], F32, tag="bb", bufs=4, name="bb")
              for _ in range(HEADS)]
        hn = [WK.tile([64, TSL], BF16, tag=f"hn{h}", bufs=1, name=f"hn{h}")
              for h in range(HEADS)]

        def row_bcast_ap(sl):
            # [1, TSL] slice -> [1, D, TSL] AP that re-reads the row D times
            return bass.AP(tensor=sl.tensor, offset=sl.offset,
                           ap=[list(sl.ap[0]), [0, D], [1, TSL]])

        # r = exp(-ln(rowsum)); batch all Ln then all Exp (table sets)
        for h in range(HEADS):
            nc.scalar.activation(out=rs[h][D:D + 1, :], in_=hp[h][D:D + 1, :],
                                 func=AF.Ln)
        for h in range(HEADS):
            nc.scalar.activation(out=rs[h][D:D + 1, :], in_=rs[h][D:D + 1, :],
                                 func=AF.Exp, scale=-1.0)
        for h in range(HEADS):
            nc.sync.dma_start(out=bb[h], in_=row_bcast_ap(rs[h][D:D + 1, :]))
            nc.vector.tensor_tensor(out=hn[h], in0=hp[h][0:D, :], in1=bb[h],
                                    op=OP.mult)

        outsb = [CP.tile([128, TSL], F32, tag=f"o{ob}", name=f"o{ob}")
                 for ob in range(2)]
        for ob in range(2):
            ops = PS.tile([128, TSL], F32, tag="S", bufs=2, name="S")
            for h in range(HEADS):
                nc.tensor.matmul(ops, lhsT=wps4[h][:, 128 * ob:128 * (ob + 1)],
                                 rhs=hn[h], start=(h == 0), stop=(h == HEADS - 1))
            # out = proj + B* + x_residual
            nc.vector.scalar_tensor_tensor(out=outsb[ob], in0=ops,
                                           scalar=bstar[:, ob:ob + 1],
                                           in1=xq[ob], op0=OP.add, op1=OP.add)
            nc.sync.dma_start(out=out_d[ob], in_=outsb[ob])


_CACHE = {}


def _get_module():
    if "nc" not in _CACHE:
        _CACHE["nc"] = _build()
    return _CACHE["nc"]


def kernel(x, gn_gamma, gn_beta, w_qkv, w_proj, b_proj):
    x = np.ascontiguousarray(np.asarray(x, dtype=np.float32))
    gn_gamma = np.asarray(gn_gamma, dtype=np.float32)
    gn_beta = np.asarray(gn_beta, dtype=np.float32)
    w_qkv = np.asarray(w_qkv, dtype=np.float32)
    w_proj = np.asarray(w_proj, dtype=np.float32)
    b_proj = np.asarray(b_proj, dtype=np.float32)

    B, Cc, H, W, Dd = x.shape
    x2 = x.reshape(Cc, H * W * Dd)

    # reference splits qkv per head: rows [192h,192h+64) = q_h, then k_h, v_h.
    # Permute to [q_all | k_all | v_all] with head-major 64-row blocks.
    perm = np.concatenate([np.arange(4) * 192 + 64 * p + np.arange(64)[:, None]
                           for p in range(3)], axis=1).T.reshape(-1)
    wqkvT = np.ascontiguousarray(w_qkv.T[:, perm]).copy()
    wqkvT[:, 0:C] *= 1.0 / np.sqrt(float(D))   # fold logit scale into q
    wprojT = np.ascontiguousarray(w_proj.T)

    base = {
        "x2": np.ascontiguousarray(x2.reshape(2, 128, N)),
        "wqkvT": np.ascontiguousarray(wqkvT.reshape(2, 128, 3 * C)),
        "wprojT": np.ascontiguousarray(wprojT.reshape(2, 128, C)),
        "gamma_col": np.ascontiguousarray(gn_gamma.reshape(2, 128, 1)),
        "beta_col": np.ascontiguousarray(gn_beta.reshape(2, 128, 1)),
        "bproj_col": np.ascontiguousarray(b_proj.reshape(2, 128, 1)),
    }
    in_maps = []
    for i in range(NCORES):
        m = dict(base)
        m["xq"] = np.ascontiguousarray(
            x2[:, i * TSL:(i + 1) * TSL].reshape(2, 128, TSL))
        in_maps.append(m)

    nc = _get_module()
    res = run_bass_kernel_spmd(nc, in_maps, core_ids=list(range(NCORES)),
                               **_CACHE.get("run_kwargs", {}))
    _CACHE["last_result"] = res
    out = np.concatenate(
        [res.results[i]["out"].reshape(C, TSL) for i in range(NCORES)], axis=1)
    return out.reshape(B, Cc, H, W, Dd).astype(np.float32)


# revision 7
# speedup vs baseline: 1.1038x; 1.1038x over previous
"""Fused GroupNorm + attention block for Trainium2 (8 NeuronCores, SPMD).

Strategy:
  - Each core computes the full output for 1/8 of the spatial positions
    (a 512-column slice of the flattened [C=256, N=4096] activation).
  - GroupNorm is folded into the QKV weights on-device:
      qkv = (W diag(a)) x + W b~   with a[c] = rstd[g(c)]*gamma[c],
      b~[c] = beta[c] - mean[g(c)]*a[c].
    The attention logit scale (1/8) is folded into the q columns of W^T
    on the host. The k bias drops out of softmax exactly; the v bias is
    recovered through the softmax row-sums and folded into the output
    bias B* = b_proj + W_p beta_v.
  - Attention is computed in "S^T" layout: S^T[s,t] = sum_d k[d,s] q[d,t]
    so the exp'd probabilities feed the AV matmul directly (contraction
    over s on the partition axis, no transposes). Row-sums come from an
    extra all-ones column appended to v^T.
  - Hot-path matmuls run in bf16 (activations are O(1) after the GN fold,
    measured end-to-end error ~3e-4 relative).
"""

import numpy as np

import concourse.bass as bass
import concourse.bacc as bacc
import concourse.tile as tile
import concourse.mybir as mybir
from concourse.bass_utils import run_bass_kernel_spmd

F32 = mybir.dt.float32
BF16 = mybir.dt.bfloat16
I32 = mybir.dt.int32
AF = mybir.ActivationFunctionType
OP = mybir.AluOpType

C = 256
N = 4096
NCORES = 8
TSL = N // NCORES          # 512 spatial positions per core
HEADS = 4
D = 64                     # head dim
NG = 16                    # groupnorm groups
GS = C // NG               # channels per group
EPS = 1e-5
NSB = N // 128             # 32 s-blocks
VW = D + 1                 # v^T columns per head incl. ones column


def _build():
    nc = bacc.Bacc("TRN2", target_bir_lowering=False, debug=False,
                   num_devices=NCORES)

    x_d = nc.dram_tensor("x2", [2, 128, N], F32, kind="ExternalInput")
    xq_d = nc.dram_tensor("xq", [2, 128, TSL], F32, kind="ExternalInput")
    wqkvT_d = nc.dram_tensor("wqkvT", [2, 128, 3 * C], F32, kind="ExternalInput")
    wprojT_d = nc.dram_tensor("wprojT", [2, 128, C], F32, kind="ExternalInput")
    gamma_d = nc.dram_tensor("gamma_col", [2, 128, 1], F32, kind="ExternalInput")
    beta_d = nc.dram_tensor("beta_col", [2, 128, 1], F32, kind="ExternalInput")
    bproj_d = nc.dram_tensor("bproj_col", [2, 128, 1], F32, kind="ExternalInput")
    out_d = nc.dram_tensor("out", [2, 128, TSL], F32, kind="ExternalOutput")

    with tile.TileContext(nc) as tc:
        _emit(nc, tc, x_d, xq_d, wqkvT_d, wprojT_d, gamma_d, beta_d,
              bproj_d, out_d)
    nc.finalize()
    return nc


def _emit(nc, tc, x_d, xq_d, wqkvT_d, wprojT_d, gamma_d, beta_d, bproj_d,
          out_d):
    import contextlib
    ctx = contextlib.ExitStack()
    with ctx:
        CP = ctx.enter_context(tc.tile_pool(name="const", bufs=1))
        WK = ctx.enter_context(tc.tile_pool(name="work", bufs=2))
        PS = ctx.enter_context(tc.tile_pool(name="psum", bufs=1, space="PSUM"))
        PPool = ctx.enter_context(tc.tile_pool(name="ptiles", bufs=3))

        # ---------------- loads (x in 8 pieces for stats pipelining) ------
        # x pieces: [c-tile][p] each [128, 1024], p = quarter of the N axis
        xch = [[CP.tile([128, N // 4], F32, tag=f"x{ct}{p}", name=f"x{ct}{p}")
                for p in range(4)] for ct in range(2)]
        xb = [[CP.tile([128, N // 4], BF16, tag=f"xb{ct}{p}", name=f"xb{ct}{p}")
               for p in range(4)] for ct in range(2)]
        stats = [WK.tile([128, 8, 6], F32, tag=f"bnstats{ct}", bufs=1,
                         name=f"bnstats{ct}") for ct in range(2)]
        for p in range(4):
            for ct in range(2):
                nc.sync.dma_start(out=xch[ct][p],
                                  in_=x_d[ct, :, p * (N // 4):(p + 1) * (N // 4)])
        xq = [CP.tile([128, TSL], F32, tag=f"xq{ct}", name=f"xq{ct}")
              for ct in range(2)]
        xqb = [CP.tile([128, TSL], BF16, tag=f"xqb{ct}", name=f"xqb{ct}")
               for ct in range(2)]
        wqkvT = [CP.tile([128, 3 * C], F32, tag=f"wq{ct}", name=f"wq{ct}")
                 for ct in range(2)]
        wprojT = [CP.tile([128, C], F32, tag=f"wp{ct}", name=f"wp{ct}")
                  for ct in range(2)]
        gcol = [CP.tile([128, 1], F32, tag=f"g{ct}", name=f"g{ct}") for ct in range(2)]
        bcol = [CP.tile([128, 1], F32, tag=f"b{ct}", name=f"b{ct}") for ct in range(2)]
        bpcol = [CP.tile([128, 1], F32, tag=f"bp{ct}", name=f"bp{ct}") for ct in range(2)]
        for ct in range(2):
            nc.sync.dma_start(out=xq[ct], in_=xq_d[ct])
            nc.sync.dma_start(out=wqkvT[ct], in_=wqkvT_d[ct])
            nc.sync.dma_start(out=wprojT[ct], in_=wprojT_d[ct])
            nc.sync.dma_start(out=gcol[ct], in_=gamma_d[ct])
            nc.sync.dma_start(out=bcol[ct], in_=beta_d[ct])
            nc.sync.dma_start(out=bpcol[ct], in_=bproj_d[ct])

        # per-piece: bn_stats (DVE) + bf16 cast (ACT), overlapping the DMAs
        for p in range(4):
            for ct in range(2):
                xv = xch[ct][p].rearrange("q (j f) -> q j f", f=512)
                for j in range(2):
                    nc.vector.bn_stats(out=stats[ct][:, 2 * p + j, :],
                                       in_=xv[:, j, :])
                nc.scalar.copy(out=xb[ct][p], in_=xch[ct][p])
        for ct in range(2):
            nc.scalar.copy(out=xqb[ct], in_=xq[ct])

        # ---------------- groupnorm statistics ----------------
        # indicator matrices built from iota + shift + is_equal
        gci = WK.tile([128, 1], I32, tag="gci", bufs=1, name="gci")
        nc.gpsimd.iota(gci, pattern=[[0, 1]], base=0, channel_multiplier=1)
        nc.vector.tensor_scalar(out=gci, in0=gci, scalar1=4, scalar2=None,
                                op0=OP.arith_shift_right)
        gcf = WK.tile([128, 1], F32, tag="gcf", bufs=1, name="gcf")
        nc.vector.tensor_copy(out=gcf, in_=gci)
        ji = WK.tile([128, 8], I32, tag="ji", bufs=1, name="ji")
        nc.gpsimd.iota(ji, pattern=[[1, 8]], base=0, channel_multiplier=0)
        jf = WK.tile([128, 8], F32, tag="jf", bufs=1, name="jf")
        nc.vector.tensor_copy(out=jf, in_=ji)
        G = CP.tile([128, 8], F32, tag="G", name="G")
        nc.vector.tensor_scalar(out=G, in0=jf, scalar1=gcf, scalar2=None,
                                op0=OP.is_equal)
        ci = WK.tile([8, 128], I32, tag="ci", bufs=1, name="ci")
        nc.gpsimd.iota(ci, pattern=[[1, 128]], base=0, channel_multiplier=0)
        nc.vector.tensor_scalar(out=ci, in0=ci, scalar1=4, scalar2=None,
                                op0=OP.arith_shift_right)
        cf = WK.tile([8, 128], F32, tag="cf", bufs=1, name="cf")
        nc.vector.tensor_copy(out=cf, in_=ci)
        gi = WK.tile([8, 1], I32, tag="gi", bufs=1, name="gi")
        nc.gpsimd.iota(gi, pattern=[[0, 1]], base=0, channel_multiplier=1)
        gif = WK.tile([8, 1], F32, tag="gif", bufs=1, name="gif")
        nc.vector.tensor_copy(out=gif, in_=gi)
        Gt = CP.tile([8, 128], F32, tag="Gt", name="Gt")
        nc.vector.tensor_scalar(out=Gt, in0=cf, scalar1=gif, scalar2=None,
                                op0=OP.is_equal)
        eps8 = CP.tile([8, 1], F32, tag="eps8", name="eps8")
        nc.vector.memset(eps8, EPS)
        ones16 = CP.tile([128, 16], F32, tag="ones16", name="ones16")
        nc.vector.memset(ones16, 1.0)

        # per-channel packed [mean, E[x^2]]
        mvp = [CP.tile([128, 2], F32, tag=f"mvp{ct}", name=f"mvp{ct}")
               for ct in range(2)]
        for ct in range(2):
            mv = WK.tile([128, 2], F32, tag="bnaggr", bufs=2, name="bnaggr")
            nc.vector.bn_aggr(out=mv, in_=stats[ct])
            nc.vector.tensor_copy(out=mvp[ct][:, 0:1], in_=mv[:, 0:1])
            nc.vector.tensor_tensor(out=mvp[ct][:, 1:2], in0=mv[:, 0:1],
                                    in1=mv[:, 0:1], op=OP.mult)
            nc.vector.tensor_tensor(out=mvp[ct][:, 1:2], in0=mvp[ct][:, 1:2],
                                    in1=mv[:, 1:2], op=OP.add)

        # group sums via G^T @ mvp -> gg[g, 2*ct + j]
        gg = PS.tile([8, 4], F32, tag="S", bufs=2, name="S")
        for ct in range(2):
            nc.tensor.matmul(gg[:, 2 * ct:2 * ct + 2], lhsT=G, rhs=mvp[ct],
                             start=(ct == 0), stop=(ct == 1))
        ggv = gg.rearrange("p (ct two) -> p ct two", two=2)
        meanL = CP.tile([8, 2], F32, tag="meanL", name="meanL")
        rstd = CP.tile([8, 2], F32, tag="rstd", name="rstd")
        tmp8 = WK.tile([8, 2], F32, tag="tmp8", name="tmp8")
        nc.vector.tensor_scalar(out=meanL, in0=ggv[:, :, 0], scalar1=1.0 / GS,
                                scalar2=None, op0=OP.mult)
        nc.vector.tensor_scalar(out=tmp8, in0=ggv[:, :, 1], scalar1=1.0 / GS,
                                scalar2=None, op0=OP.mult)
        nc.vector.tensor_tensor(out=rstd, in0=meanL, in1=meanL, op=OP.mult)
        nc.vector.tensor_tensor(out=rstd, in0=tmp8, in1=rstd, op=OP.subtract)
        # rstd = exp(-0.5 * ln(var + eps))
        nc.scalar.activation(out=rstd, in_=rstd, func=AF.Ln, bias=eps8)
        nc.scalar.activation(out=rstd, in_=rstd, func=AF.Exp, scale=-0.5)

        # expand group values to channels and build a, b~
        acol = [CP.tile([128, 1], F32, tag=f"acol{ct}", name=f"acol{ct}")
                for ct in range(2)]
        btcol = [CP.tile([128, 1], F32, tag=f"btcol{ct}", name=f"btcol{ct}")
                 for ct in range(2)]
        for ct in range(2):
            rexp = PS.tile([128, 1], F32, tag="S", bufs=2, name="S")
            nc.tensor.matmul(rexp, lhsT=Gt, rhs=rstd[:, ct:ct + 1],
                             start=True, stop=True)
            mexp = PS.tile([128, 1], F32, tag="S", bufs=2, name="S")
            nc.tensor.matmul(mexp, lhsT=Gt, rhs=meanL[:, ct:ct + 1],
                             start=True, stop=True)
            nc.vector.tensor_tensor(out=acol[ct], in0=rexp, in1=gcol[ct],
                                    op=OP.mult)
            nc.vector.tensor_tensor(out=btcol[ct], in0=mexp, in1=acol[ct],
                                    op=OP.mult)
            nc.vector.tensor_tensor(out=btcol[ct], in0=bcol[ct], in1=btcol[ct],
                                    op=OP.subtract)

        # scaled weights W'^T = W^T * a (per-partition), bf16
        wqs = [CP.tile([128, 3 * C], BF16, tag=f"wqs{ct}", name=f"wqs{ct}")
               for ct in range(2)]
        for ct in range(2):
            nc.vector.tensor_scalar_mul(out=wqs[ct], in0=wqkvT[ct],
                                        scalar1=acol[ct])

        # qkv bias beta = W^T.T @ b~  (q blocks 0,1 and v blocks 4,5)
        betaq = CP.tile([128, 2], F32, tag="betaq", name="betaq")
        betav = CP.tile([128, 2], F32, tag="betav", name="betav")
        for i, ob in enumerate((0, 1, 4, 5)):
            bps = PS.tile([128, 1], F32, tag="S", bufs=2, name="S")
            for ct in range(2):
                nc.tensor.matmul(bps, lhsT=wqkvT[ct][:, 128 * ob:128 * (ob + 1)],
                                 rhs=btcol[ct], start=(ct == 0), stop=(ct == 1))
            dst = betaq if ob < 2 else betav
            nc.vector.tensor_copy(out=dst[:, i % 2:i % 2 + 1], in_=bps)

        # B* = b_proj + W_p @ beta_v
        bstar = CP.tile([128, 2], F32, tag="bstar", name="bstar")
        for ob in range(2):
            bps = PS.tile([128, 1], F32, tag="S", bufs=2, name="S")
            for ct in range(2):
                nc.tensor.matmul(bps,
                                 lhsT=wprojT[ct][:, 128 * ob:128 * (ob + 1)],
                                 rhs=betav[:, ct:ct + 1],
                                 start=(ct == 0), stop=(ct == 1))
            nc.vector.tensor_tensor(out=bstar[:, ob:ob + 1], in0=bps,
                                    in1=bpcol[ob], op=OP.add)

        # per-head projection weights at partitions 0-63 (DMA partition shift)
        wps4 = [CP.tile([64, C], BF16, tag=f"wps4_{h}", name=f"wps4_{h}")
                for h in range(HEADS)]
        wpsb = [CP.tile([128, C], BF16, tag=f"wpsb{ct}", name=f"wpsb{ct}")
                for ct in range(2)]
        for ct in range(2):
            nc.vector.tensor_copy(out=wpsb[ct], in_=wprojT[ct])
        for h in range(HEADS):
            if h % 2 == 0:
                nc.vector.tensor_copy(out=wps4[h], in_=wpsb[h // 2][0:64, :])
            else:
                nc.sync.dma_start(out=wps4[h], in_=wpsb[h // 2][64:128, :])

        # ---------------- q (this core's slice) ----------------
        q = [CP.tile([128, TSL], BF16, tag=f"q{ob}", name=f"q{ob}")
             for ob in range(2)]
        for ob in range(2):
            qps = PS.tile([128, TSL], F32, tag="S", bufs=2, name="S")
            for ct in range(2):
                nc.tensor.matmul(qps,
                                 lhsT=wqs[ct][:, 128 * ob:128 * (ob + 1)],
                                 rhs=xqb[ct], start=(ct == 0), stop=(ct == 1))
            nc.vector.tensor_scalar(out=q[ob], in0=qps,
                                    scalar1=betaq[:, ob:ob + 1], scalar2=None,
                                    op0=OP.add)

        # ---------------- k, v^T production helpers ----------------
        kc = [[CP.tile([128, 512], BF16, tag=f"k{ob}_{j}", name=f"k{ob}_{j}")
               for j in range(8)] for ob in range(2)]
        vt = [CP.tile([128, 4, 4 * VW], BF16, tag=f"vt{j}", name=f"vt{j}")
              for j in range(8)]

        def xb_slice(j, ct, width, off=0):
            p = j // 2
            col = 512 * (j % 2) + off
            return xb[ct][p][:, col:col + width]

        def produce_k(j, ob, pool, tag):
            kps = pool.tile([128, 512], F32, tag=tag, bufs=1, name="kps")
            for ct in range(2):
                nc.tensor.matmul(
                    kps,
                    lhsT=wqs[ct][:, C + 128 * ob: C + 128 * (ob + 1)],
                    rhs=xb_slice(j, ct, 512), start=(ct == 0), stop=(ct == 1))
            nc.vector.tensor_copy(out=kc[ob][j], in_=kps)

        def produce_v(j, pool, tag):
            vps = pool.tile([128, 4, C], F32, tag=tag, bufs=1, name="vps")
            for jj in range(4):
                for ct in range(2):
                    nc.tensor.matmul(
                        vps[:, jj, :],
                        lhsT=xb_slice(j, ct, 128, off=128 * jj),
                        rhs=wqs[ct][:, 2 * C:3 * C],
                        start=(ct == 0), stop=(ct == 1))
            vtv = vt[j].rearrange("p j (h w) -> p j h w", w=VW)
            nc.vector.tensor_copy(
                out=vtv[:, :, :, 0:D],
                in_=vps.rearrange("p j (h d) -> p j h d", d=D))
            nc.vector.tensor_copy(
                out=vtv[:, :, :, D:VW],
                in_=ones16.rearrange("p (j h w) -> p j h w", h=4, w=1))

        def attention_step(pair, j, jj, sb):
            sps = PS.tile([128, 2 * TSL], F32, tag="S", bufs=2, name="S")
            for po in range(2):
                base = 64 * po
                nc.tensor.matmul(
                    sps[:, TSL * po:TSL * (po + 1)],
                    lhsT=kc[pair][j][base:base + 64, 128 * jj:128 * (jj + 1)],
                    rhs=q[pair][base:base + 64, :],
                    start=True, stop=True)
            pt = PPool.tile([128, 2 * TSL], BF16, tag="P", name="P")
            nc.scalar.activation(out=pt, in_=sps, func=AF.Exp)
            for po in range(2):
                h = 2 * pair + po
                nc.tensor.matmul(
                    hp[h],
                    lhsT=vt[j][:, jj, VW * h:VW * (h + 1)],
                    rhs=pt[:, TSL * po:TSL * (po + 1)],
                    start=(sb == 0), stop=(sb == NSB - 1))

        hp = [None] * HEADS
        hp[0] = PS.tile([VW, TSL], F32, tag="h0", name="h0")
        hp[1] = PS.tile([VW, TSL], F32, tag="h1", name="h1")

        rs = [WK.tile([128, TSL], F32, tag=f"rs{h}", bufs=1, name=f"rs{h}")
              for h in range(HEADS)]
        bb = [WK.tile([64, TSL], F32, tag=f"bb{h}", bufs=1, name=f"bb{h}")
              for h in range(HEADS)]
        hn = [WK.tile([64, TSL], BF16, tag=f"hn{h}", bufs=1, name=f"hn{h}")
              for h in range(HEADS)]

        def row_bcast_ap(sl):
            # [1, TSL] slice -> [1, D, TSL] AP that re-reads the row D times
            return bass.AP(tensor=sl.tensor, offset=sl.offset,
                           ap=[list(sl.ap[0]), [0, D], [1, TSL]])

        # ---------------- pass 1: heads 0,1 (+ all k/v production) --------
        with tc.tile_pool(name="prod", bufs=1, space="PSUM") as PROD:
            produce_k(0, 0, PROD, "prod")
            produce_v(0, PROD, "prod")
            for j in range(8):
                for jj in range(4):
                    attention_step(0, j, jj, 4 * j + jj)
                    if jj == 1 and j < 7:
                        produce_k(j + 1, 0, PROD, "prod")
                    elif jj == 2 and j < 7:
                        produce_v(j + 1, PROD, "prod")
                    elif jj == 3 and j >= 6:
                        produce_k(j - 6, 1, PROD, "prod")

        # normalization of heads 0,1 overlaps pass 2 (DVE reciprocal — the
        # ACT tables stay on exp)
        for h in range(2):
            nc.vector.reciprocal(out=rs[h][D:D + 1, :], in_=hp[h][D:D + 1, :])
            nc.sync.dma_start(out=bb[h], in_=row_bcast_ap(rs[h][D:D + 1, :]))
            nc.vector.tensor_tensor(out=hn[h], in0=hp[h][0:D, :], in1=bb[h],
                                    op=OP.mult)

        # ---------------- pass 2: heads 2,3 ----------------
        with tc.tile_pool(name="psB", bufs=1, space="PSUM") as PSB:
            hp[2] = PSB.tile([VW, TSL], F32, tag="h2", name="h2")
            hp[3] = PSB.tile([VW, TSL], F32, tag="h3", name="h3")
            for j in range(8):
                for jj in range(4):
                    attention_step(1, j, jj, 4 * j + jj)
                    # late k production re-uses the freed h0/h1 banks
                    if jj == 1 and j < 6:
                        produce_k(j + 2, 1, PS, f"h{j % 2}")

            # ---------------- tail: heads 2,3 + projection ----------------
            for h in (2, 3):
                nc.scalar.activation(out=rs[h][D:D + 1, :],
                                     in_=hp[h][D:D + 1, :], func=AF.Ln)
            for h in (2, 3):
                nc.scalar.activation(out=rs[h][D:D + 1, :],
                                     in_=rs[h][D:D + 1, :], func=AF.Exp,
                                     scale=-1.0)
            for h in (2, 3):
                nc.sync.dma_start(out=bb[h], in_=row_bcast_ap(rs[h][D:D + 1, :]))
                nc.vector.tensor_tensor(out=hn[h], in0=hp[h][0:D, :],
                                        in1=bb[h], op=OP.mult)

            outsb = [CP.tile([128, TSL], F32, tag=f"o{ob}", name=f"o{ob}")
                     for ob in range(2)]
            for ob in range(2):
                ops = PS.tile([128, TSL], F32, tag="S", bufs=2, name="S")
                for h in range(HEADS):
                    nc.tensor.matmul(ops,
                                     lhsT=wps4[h][:, 128 * ob:128 * (ob + 1)],
                                     rhs=hn[h], start=(h == 0),
                                     stop=(h == HEADS - 1))
                nc.vector.scalar_tensor_tensor(out=outsb[ob], in0=ops,
                                               scalar=bstar[:, ob:ob + 1],
                                               in1=xq[ob], op0=OP.add,
                                               op1=OP.add)
                nc.sync.dma_start(out=out_d[ob], in_=outsb[ob])


_CACHE = {}


def _get_module():
    if "nc" not in _CACHE:
        _CACHE["nc"] = _build()
    return _CACHE["nc"]


def kernel(x, gn_gamma, gn_beta, w_qkv, w_proj, b_proj):
    x = np.ascontiguousarray(np.asarray(x, dtype=np.float32))
    gn_gamma = np.asarray(gn_gamma, dtype=np.float32)
    gn_beta = np.asarray(gn_beta, dtype=np.float32)
    w_qkv = np.asarray(w_qkv, dtype=np.float32)
    w_proj = np.asarray(w_proj, dtype=np.float32)
    b_proj = np.asarray(b_proj, dtype=np.float32)

    B, Cc, H, W, Dd = x.shape
    x2 = x.reshape(Cc, H * W * Dd)

    # reference splits qkv per head: rows [192h,192h+64) = q_h, then k_h, v_h.
    # Permute to [q_all | k_all | v_all] with head-major 64-row blocks.
    perm = np.concatenate([np.arange(4) * 192 + 64 * p + np.arange(64)[:, None]
                           for p in range(3)], axis=1).T.reshape(-1)
    wqkvT = np.ascontiguousarray(w_qkv.T[:, perm]).copy()
    wqkvT[:, 0:C] *= 1.0 / np.sqrt(float(D))   # fold logit scale into q
    wprojT = np.ascontiguousarray(w_proj.T)

    base = {
        "x2": np.ascontiguousarray(x2.reshape(2, 128, N)),
        "wqkvT": np.ascontiguousarray(wqkvT.reshape(2, 128, 3 * C)),
        "wprojT": np.ascontiguousarray(wprojT.reshape(2, 128, C)),
        "gamma_col": np.ascontiguousarray(gn_gamma.reshape(2, 128, 1)),
        "beta_col": np.ascontiguousarray(gn_beta.reshape(2, 128, 1)),
        "bproj_col": np.ascontiguousarray(b_proj.reshape(2, 128, 1)),
    }
    in_maps = []
    for i in range(NCORES):
        m = dict(base)
        m["xq"] = np.ascontiguousarray(
            x2[:, i * TSL:(i + 1) * TSL].reshape(2, 128, TSL))
        in_maps.append(m)

    nc = _get_module()
    res = run_bass_kernel_spmd(nc, in_maps, core_ids=list(range(NCORES)),
                               **_CACHE.get("run_kwargs", {}))
    _CACHE["last_result"] = res
    out = np.concatenate(
        [res.results[i]["out"].reshape(C, TSL) for i in range(NCORES)], axis=1)
    return out.reshape(B, Cc, H, W, Dd).astype(np.float32)
